# revision 6
# baseline (speedup 1.0000x reference)
"""Trainium2 Bass kernel: segmented-softmax weighted normalization.

Math (all weights positive, so sum|w| == sum w):
  g = feats @ w_global + b_g ;  l = feats @ w_local + b_l     (per row)
  u = sigmoid(l) * exp(g) ;  e = exp(g)
  per segment b: A[b,:] = sum u*f ; B[b,:] = sum u*f^2 ; s[b] = sum u ; z[b] = sum e
  (exp without max-subtraction: g ~ N(0,1), safe in f32/bf16)
  S = sum_b s[b]/z[b] ; mean = (sum_b A[b,:]/z[b]) / S ; E2 = (sum_b B[b,:]/z[b]) / S
  std = sqrt(E2 - mean^2) ;  out = f*rstd - mean*rstd

Distribution: shard N rows over 8 cores; ONE AllReduce of the [16, 514]
per-core partials (A|B|s|z).  Segment raggedness handled by a one-hot
matrix H[row, b] built on-device from segment ids, folded into per-tile
PE matmuls that contract the 128-row partition axis.

Two HBM passes over feats (stats, then normalize) + one output write
= 1.5 GB total traffic across 8 cores.
"""
import sys

sys.path.insert(0, "/opt/trn_rl_repo")
import numpy as np
import ml_dtypes

import concourse.bass as bass
import concourse.tile as tile
from concourse import bacc, mybir
from concourse.bass_utils import run_bass_kernel_spmd

F32 = mybir.dt.float32
BF16 = mybir.dt.bfloat16
P = 128
B = 16          # segments
C = 256         # channels
NCORES = 8
T = 8           # row-tiles per macro-tile
GL_BOUNCE = "act"   # "dma" or "act": how [2, T*128] matvec psum reaches sbuf


def build_graph(tiles: int, trace_friendly: bool = False):
    """One SPMD graph for all 8 cores; `tiles` 128-row tiles per core."""
    assert tiles % T == 0
    macros = tiles // T
    nc = bacc.Bacc("TRN2", target_bir_lowering=False, debug=False,
                   num_devices=NCORES)

    feats_d = nc.dram_tensor("feats", [tiles, P, C], F32, kind="ExternalInput")
    segs_d = nc.dram_tensor("segs", [P, tiles], BF16, kind="ExternalInput")
    wsb_d = nc.dram_tensor("wsb", [P, 4], BF16, kind="ExternalInput")
    bias_d = nc.dram_tensor("bias2", [P, 2], F32, kind="ExternalInput")
    identb_d = nc.dram_tensor("identb", [P, P], BF16, kind="ExternalInput")
    identf_d = nc.dram_tensor("identf", [2, 2], F32, kind="ExternalInput")
    iota_d = nc.dram_tensor("iotab", [P, B], BF16, kind="ExternalInput")
    onesb_d = nc.dram_tensor("onesb", [P, 1], BF16, kind="ExternalInput")
    onesf_d = nc.dram_tensor("onesf", [B, P], F32, kind="ExternalInput")
    out_d = nc.dram_tensor("out", [tiles, P, C], F32, kind="ExternalOutput")

    with tile.TileContext(nc) as tc:
        with (
            tc.tile_pool(name="const", bufs=1) as pc,
            tc.tile_pool(name="psA", bufs=1, space="PSUM") as ppA,
            tc.tile_pool(name="dram", bufs=1, space="DRAM") as pdram,
            tc.tile_pool(name="fin", bufs=1) as pfin,
        ):
            # ---- constants
            segs = pc.tile([P, tiles], BF16)
            nc.sync.dma_start(segs[:], segs_d[:])
            wsb = pc.tile([P, 4], BF16)
            nc.sync.dma_start(wsb[:], wsb_d[:])
            bias = pc.tile([P, 2], F32)
            nc.sync.dma_start(bias[:], bias_d[:])
            identb = pc.tile([P, P], BF16)
            nc.sync.dma_start(identb[:], identb_d[:])
            identf = pc.tile([2, 2], F32)
            nc.sync.dma_start(identf[:], identf_d[:])
            iota = pc.tile([P, B], BF16)
            nc.sync.dma_start(iota[:], iota_d[:])
            onesb = pc.tile([P, 1], BF16)
            nc.sync.dma_start(onesb[:], onesb_d[:])
            onesf = pc.tile([B, P], F32)
            nc.sync.dma_start(onesf[:], onesf_d[:])

            # ---- persistent psum accumulators: A|B [16, 512], s|z [16, 2]
            ppAB = ppA.tile([B, 2 * C], F32)
            ppS = ppA.tile([B, 2], F32, tag="ppS")

            # =================== PASS 1: statistics ===================
            with (
                tc.tile_pool(name="pF", bufs=3) as pF,
                tc.tile_pool(name="pFb", bufs=2) as pFb,
                tc.tile_pool(name="pFT", bufs=2) as pFT,
                tc.tile_pool(name="pGl", bufs=2) as pGl,
                tc.tile_pool(name="pSm", bufs=2) as pSm,
                tc.tile_pool(name="ppFT", bufs=2, space="PSUM") as ppFT,
                tc.tile_pool(name="ppGl", bufs=1, space="PSUM") as ppGl,
                tc.tile_pool(name="ppGt", bufs=1, space="PSUM") as ppGt,
            ):
                for m in range(macros):
                    f_t = pF.tile([P, T, C], F32)
                    nc.sync.dma_start(
                        f_t[:], feats_d[m * T:(m + 1) * T].rearrange("t p c -> p t c"))

                    fb = pFb.tile([P, T, C], BF16, tag="fb")
                    nc.vector.tensor_copy(fb[:], f_t[:])
                    f2b = pFb.tile([P, T, C], BF16, tag="f2b")
                    nc.vector.tensor_tensor(f2b[:], fb[:], fb[:], mybir.AluOpType.mult)

                    # transpose fb -> fT [128ch(half), 2(half), T*128 rows]
                    fT = pFT.tile([P, 2, T * P], BF16)
                    for g4 in range(T // 2):
                        pt = ppFT.tile([P, 2, 2, P], BF16)
                        for tt in range(2):
                            for h in range(2):
                                nc.tensor.transpose(
                                    pt[:, tt, h, :],
                                    fb[:, 2 * g4 + tt, h * P:(h + 1) * P],
                                    identb[:])
                        dst = fT[:, :, 2 * P * g4: 2 * P * (g4 + 1)]
                        dst = dst.rearrange("p h (tt r) -> p tt h r", tt=2)
                        nc.scalar.activation(dst, pt[:],
                                             mybir.ActivationFunctionType.Copy)

                    # matvec: gl[2, T*128] += w_half.T @ fT_half
                    glp = ppGl.tile([2, T * P], F32)
                    for rc in range(T * P // 512):
                        for h in range(2):
                            nc.tensor.matmul(
                                glp[:, rc * 512:(rc + 1) * 512],
                                wsb[:, 2 * h:2 * h + 2],
                                fT[:, h, rc * 512:(rc + 1) * 512],
                                start=(h == 0), stop=(h == 1))

                    gl_sb = pGl.tile([2, T * P], F32)
                    if GL_BOUNCE == "dma":
                        nc.sync.dma_start(gl_sb[:], glp[:])
                    else:
                        nc.scalar.activation(gl_sb[:], glp[:],
                                             mybir.ActivationFunctionType.Copy)

                    # transpose gl -> [128, T, 2]
                    glt = ppGt.tile([P, T, 2], F32)
                    for t in range(T):
                        nc.tensor.transpose(glt[:, t, :],
                                            gl_sb[:, t * P:(t + 1) * P],
                                            identf[:])

                    e_bf = pSm.tile([P, T], BF16, tag="e")
                    nc.scalar.activation(e_bf[:], glt[:, :, 0],
                                         mybir.ActivationFunctionType.Exp,
                                         bias=bias[:, 0:1])
                    sg_bf = pSm.tile([P, T], BF16, tag="sg")
                    nc.scalar.activation(sg_bf[:], glt[:, :, 1],
                                         mybir.ActivationFunctionType.Sigmoid,
                                         bias=bias[:, 1:2])
                    u_bf = pSm.tile([P, T], BF16, tag="u")
                    nc.vector.tensor_tensor(u_bf[:], sg_bf[:], e_bf[:],
                                            mybir.AluOpType.mult)

                    H = pSm.tile([P, T, B], BF16, tag="H")
                    nc.vector.tensor_tensor(
                        H[:],
                        segs[:, m * T:(m + 1) * T].unsqueeze(2).to_broadcast((P, T, B)),
                        iota[:].unsqueeze(1).to_broadcast((P, T, B)),
                        mybir.AluOpType.is_equal)
                    Hu = pSm.tile([P, T, B], BF16, tag="Hu")
                    nc.vector.tensor_tensor(
                        Hu[:], H[:],
                        u_bf[:].unsqueeze(2).to_broadcast((P, T, B)),
                        mybir.AluOpType.mult)

                    for t in range(T):
                        tg = m * T + t
                        st = (tg == 0)
                        sp = (tg == tiles - 1)
                        nc.tensor.matmul(ppAB[:, 0:C], Hu[:, t, :], fb[:, t, :],
                                         start=st, stop=sp, skip_group_check=True)
                        nc.tensor.matmul(ppAB[:, C:2 * C], Hu[:, t, :], f2b[:, t, :],
                                         start=st, stop=sp, skip_group_check=True)
                        nc.tensor.matmul(ppS[:, 0:1], Hu[:, t, :], onesb[:],
                                         start=st, stop=sp, skip_group_check=True)
                        nc.tensor.matmul(ppS[:, 1:2], H[:, t, :], e_bf[:, t:t + 1],
                                         start=st, stop=sp, skip_group_check=True)

            # =================== collective + finals ===================
            with (
                tc.tile_pool(name="ep", bufs=1) as pe,
                tc.tile_pool(name="ppE", bufs=1, space="PSUM") as ppE,
            ):
                ABs = pe.tile([B, 2 * C + 2], F32)
                nc.scalar.activation(ABs[:, 0:2 * C], ppAB[:],
                                     mybir.ActivationFunctionType.Copy)
                nc.scalar.activation(ABs[:, 2 * C:2 * C + 2], ppS[:],
                                     mybir.ActivationFunctionType.Copy)

                cc_in = pdram.tile([B, 2 * C + 2], F32)
                cc_out = pdram.tile([B, 2 * C + 2], F32)
                nc.sync.dma_start(cc_in[:], ABs[:])
                nc.gpsimd.collective_compute(
                    "AllReduce", mybir.AluOpType.add,
                    replica_groups=[list(range(NCORES))],
                    ins=[cc_in.opt()], outs=[cc_out.opt()])
                R = pe.tile([B, 2 * C + 2], F32)
                nc.sync.dma_start(R[:], cc_out[:])

                # rs = 1/(z+tiny) ; scale [A|B|s] rows by rs (tiny keeps
                # empty segments at 0 contribution instead of 0*inf=NaN)
                zs = pe.tile([B, 1], F32, tag="zs")
                nc.vector.tensor_scalar_add(zs[:], R[:, 2 * C + 1:2 * C + 2], 1e-30)
                rs = pe.tile([B, 1], F32)
                nc.vector.reciprocal(rs[:], zs[:])
                ABn = pe.tile([B, 2 * C + 1], F32)
                nc.vector.tensor_scalar_mul(ABn[:], R[:, 0:2 * C + 1], rs[:])

                # column-sum over the 16 segment partitions via matmul
                tot = ppE.tile([1, 2 * C], F32)
                nc.tensor.matmul(tot[0:1, :], onesf[:, 0:1], ABn[:, 0:2 * C],
                                 start=True, stop=True, skip_group_check=True)
                tot2 = ppE.tile([1, 1], F32, tag="tot2")
                nc.tensor.matmul(tot2[0:1, 0:1], onesf[:, 0:1], ABn[:, 2 * C:2 * C + 1],
                                 start=True, stop=True, skip_group_check=True)

                sinv = pe.tile([1, 1], F32)
                nc.vector.reciprocal(sinv[:], tot2[0:1, 0:1])
                fin = pe.tile([1, 2 * C], F32)   # [mean | E2]
                nc.vector.tensor_scalar_mul(fin[:], tot[0:1, :], sinv[:])

                mean2 = pe.tile([1, C], F32)
                nc.vector.tensor_tensor(mean2[:], fin[:, 0:C], fin[:, 0:C],
                                        mybir.AluOpType.mult)
                var = pe.tile([1, C], F32)
                nc.vector.tensor_tensor(var[:], fin[:, C:2 * C], mean2[:],
                                        mybir.AluOpType.subtract)
                stdv = pe.tile([1, C], F32)
                nc.scalar.activation(stdv[:], var[:],
                                     mybir.ActivationFunctionType.Sqrt)
                mr = pe.tile([1, 2 * C], F32)    # [mean*rstd | rstd]
                nc.vector.reciprocal(mr[:, C:2 * C], stdv[:])
                nc.vector.tensor_tensor(mr[:, 0:C], fin[:, 0:C], mr[:, C:2 * C],
                                        mybir.AluOpType.mult)

                # replicate [1, 512] -> [128, 512] via K=1 matmul with ones row
                rep = ppE.tile([P, 2 * C], F32, tag="rep")
                nc.tensor.matmul(rep[:], onesf[0:1, :], mr[:],
                                 start=True, stop=True, skip_group_check=True)
                mrr = pfin.tile([P, 2 * C], F32)
                nc.scalar.activation(mrr[:], rep[:],
                                     mybir.ActivationFunctionType.Copy)

            # =================== PASS 2: normalize ===================
            with (
                tc.tile_pool(name="pF2", bufs=3) as pF2,
                tc.tile_pool(name="pO", bufs=3) as pO,
            ):
                mmul_b = mrr[:, 0:C].unsqueeze(1).to_broadcast((P, T, C))
                rstd_b = mrr[:, C:2 * C].unsqueeze(1).to_broadcast((P, T, C))
                for m in range(macros):
                    f_t = pF2.tile([P, T, C], F32)
                    nc.sync.dma_start(
                        f_t[:], feats_d[m * T:(m + 1) * T].rearrange("t p c -> p t c"))
                    o1 = pO.tile([P, T, C], F32, tag="o1")
                    nc.vector.tensor_tensor(o1[:], f_t[:], rstd_b,
                                            mybir.AluOpType.mult)
                    o2 = pO.tile([P, T, C], F32, tag="o2")
                    nc.vector.tensor_tensor(o2[:], o1[:], mmul_b,
                                            mybir.AluOpType.subtract)
                    nc.sync.dma_start(
                        out_d[m * T:(m + 1) * T].rearrange("t p c -> p t c"), o2[:])

    nc.compile()
    return nc


def _prep_inputs(feats, segment_ids, w_local, b_local, w_global, b_global):
    n, c = feats.shape
    assert c == C
    rows_core = (n + NCORES - 1) // NCORES
    macros = (rows_core + T * P - 1) // (T * P)
    tiles = macros * T
    rows_pad = tiles * P

    wcat = np.concatenate([w_global.reshape(C, 1), w_local.reshape(C, 1)], axis=1)
    wsb = wcat.reshape(2, P, 2).transpose(1, 0, 2).reshape(P, 4)  # [c, 2h+j]
    bias2 = np.tile(np.array([b_global[0], b_local[0]], np.float32), (P, 1))

    in_maps = []
    for k in range(NCORES):
        lo, hi = k * rows_core, min((k + 1) * rows_core, n)
        fs = np.zeros((rows_pad, C), np.float32)
        fs[:hi - lo] = feats[lo:hi]
        ss = np.full((rows_pad,), -1.0, np.float32)
        ss[:hi - lo] = segment_ids[lo:hi].astype(np.float32)
        in_maps.append({
            "feats": np.ascontiguousarray(fs.reshape(tiles, P, C)),
            "segs": np.ascontiguousarray(
                ss.reshape(tiles, P).T.astype(ml_dtypes.bfloat16)),
            "wsb": wsb.astype(ml_dtypes.bfloat16),
            "bias2": bias2.astype(np.float32),
            "identb": np.eye(P, dtype=ml_dtypes.bfloat16),
            "identf": np.eye(2, dtype=np.float32),
            "iotab": np.tile(np.arange(B, dtype=np.float32), (P, 1)).astype(
                ml_dtypes.bfloat16),
            "onesb": np.ones((P, 1), ml_dtypes.bfloat16),
            "onesf": np.ones((B, P), np.float32),
        })
    return in_maps, tiles, rows_core


_CACHE = {}


def _run(in_maps, tiles, **kw):
    if tiles not in _CACHE:
        _CACHE[tiles] = build_graph(tiles)
    nc = _CACHE[tiles]
    return run_bass_kernel_spmd(nc, in_maps, core_ids=list(range(NCORES)), **kw)


def kernel(feats, segment_ids, w_local, b_local, w_global, b_global,
           _return_results=False, **run_kw):
    feats = np.asarray(feats, np.float32)
    segment_ids = np.asarray(segment_ids)
    in_maps, tiles, rows_core = _prep_inputs(
        feats, segment_ids,
        np.asarray(w_local, np.float32), np.asarray(b_local, np.float32),
        np.asarray(w_global, np.float32), np.asarray(b_global, np.float32))
    res = _run(in_maps, tiles, **run_kw)
    n = feats.shape[0]
    outs = []
    for k in range(NCORES):
        lo, hi = k * rows_core, min((k + 1) * rows_core, n)
        o = res.results[k]["out"].reshape(-1, C)[:hi - lo]
        outs.append(o)
    full = np.concatenate(outs, axis=0)
    if _return_results:
        return full, res
    return full


# revision 9
# speedup vs baseline: 1.1635x; 1.1635x over previous
"""Trainium2 Bass kernel: segmented-softmax weighted normalization.

Math (all weights positive, so sum|w| == sum w):
  g = feats @ w_global + b_g ;  l = feats @ w_local + b_l     (per row)
  u = sigmoid(l) * exp(g) ;  e = exp(g)
  per segment b: A[b,:] = sum u*f ; B[b,:] = sum u*f^2 ; s[b] = sum u ; z[b] = sum e
  (exp without max-subtraction: g ~ N(0,1), safe in f32/bf16)
  S = sum_b s[b]/z[b] ; mean = (sum_b A[b,:]/z[b]) / S ; E2 = (sum_b B[b,:]/z[b]) / S
  std = sqrt(E2 - mean^2) ;  out = f*rstd - mean*rstd

Distribution: shard N rows over 8 cores; ONE AllReduce of the [16, 514]
per-core partials (A|B|s|z).  Segment raggedness handled by a one-hot
matrix H[row, b] built on-device from segment ids, folded into per-tile
PE matmuls that contract the 128-row partition axis.

Two HBM passes over feats (stats, then normalize) + one output write
= 1.5 GB total traffic across 8 cores.
"""
import sys

sys.path.insert(0, "/opt/trn_rl_repo")
import numpy as np
import ml_dtypes

import concourse.bass as bass
import concourse.tile as tile
from concourse import bacc, mybir
from concourse.bass_utils import run_bass_kernel_spmd

F32 = mybir.dt.float32
BF16 = mybir.dt.bfloat16
P = 128
B = 16          # segments
C = 256         # channels
NCORES = 8
T = 8           # row-tiles per macro-tile
GL_BOUNCE = "act"   # "dma" or "act": how [2, T*128] matvec psum reaches sbuf


def build_graph(tiles: int, trace_friendly: bool = False):
    """One SPMD graph for all 8 cores; `tiles` 128-row tiles per core."""
    assert tiles % T == 0
    macros = tiles // T
    nc = bacc.Bacc("TRN2", target_bir_lowering=False, debug=False,
                   num_devices=NCORES)

    feats_d = nc.dram_tensor("feats", [tiles, P, C], F32, kind="ExternalInput")
    segs_d = nc.dram_tensor("segs", [P, tiles], BF16, kind="ExternalInput")
    wsb_d = nc.dram_tensor("wsb", [P, 4], BF16, kind="ExternalInput")
    bias_d = nc.dram_tensor("bias2", [P, 2], F32, kind="ExternalInput")
    identb_d = nc.dram_tensor("identb", [P, P], BF16, kind="ExternalInput")
    identf_d = nc.dram_tensor("identf", [2, 2], F32, kind="ExternalInput")
    iota_d = nc.dram_tensor("iotab", [P, B], BF16, kind="ExternalInput")
    onesb_d = nc.dram_tensor("onesb", [P, 1], BF16, kind="ExternalInput")
    onesf_d = nc.dram_tensor("onesf", [B, P], F32, kind="ExternalInput")
    out_d = nc.dram_tensor("out", [tiles, P, C], F32, kind="ExternalOutput")

    with tile.TileContext(nc) as tc:
        with (
            tc.tile_pool(name="const", bufs=1) as pc,
            tc.tile_pool(name="psA", bufs=1, space="PSUM") as ppA,
            tc.tile_pool(name="dram", bufs=1, space="DRAM") as pdram,
            tc.tile_pool(name="fin", bufs=1) as pfin,
        ):
            # ---- constants
            segs = pc.tile([P, tiles], BF16)
            nc.sync.dma_start(segs[:], segs_d[:])
            wsb = pc.tile([P, 4], BF16)
            nc.sync.dma_start(wsb[:], wsb_d[:])
            bias = pc.tile([P, 2], F32)
            nc.sync.dma_start(bias[:], bias_d[:])
            identb = pc.tile([P, P], BF16)
            nc.sync.dma_start(identb[:], identb_d[:])
            identf = pc.tile([2, 2], F32)
            nc.sync.dma_start(identf[:], identf_d[:])
            iota = pc.tile([P, B], BF16)
            nc.sync.dma_start(iota[:], iota_d[:])
            onesb = pc.tile([P, 1], BF16)
            nc.sync.dma_start(onesb[:], onesb_d[:])
            onesf = pc.tile([B, P], F32)
            nc.sync.dma_start(onesf[:], onesf_d[:])

            # ---- persistent psum accumulators: A|B [16, 512], s|z [16, 2]
            ppAB = ppA.tile([B, 2 * C], F32)
            ppS = ppA.tile([B, 2], F32, tag="ppS")

            # =================== PASS 1: statistics ===================
            with (
                tc.tile_pool(name="pF", bufs=4) as pF,
                tc.tile_pool(name="pFb", bufs=3) as pFb,
                tc.tile_pool(name="pFT", bufs=3) as pFT,
                tc.tile_pool(name="pGl", bufs=3) as pGl,
                tc.tile_pool(name="pSm", bufs=4) as pSm,
                tc.tile_pool(name="ppFT", bufs=3, space="PSUM") as ppFT,
                tc.tile_pool(name="ppGl", bufs=1, space="PSUM") as ppGl,
                tc.tile_pool(name="ppGt", bufs=1, space="PSUM") as ppGt,
            ):
                for m in range(macros):
                    f_t = pF.tile([P, T, C], F32)
                    nc.sync.dma_start(
                        f_t[:], feats_d[m * T:(m + 1) * T].rearrange("t p c -> p t c"))

                    fb = pFb.tile([P, T, C], BF16, tag="fb")
                    nc.vector.tensor_copy(fb[:], f_t[:])
                    f2b = pFb.tile([P, T, C], BF16, tag="f2b")
                    nc.vector.tensor_tensor(f2b[:], fb[:], fb[:], mybir.AluOpType.mult)

                    # transpose fb -> fT [128ch(half), 2(half), T*128 rows]
                    fT = pFT.tile([P, 2, T * P], BF16)
                    for g4 in range(T // 2):
                        pt = ppFT.tile([P, 2, 2, P], BF16)
                        for tt in range(2):
                            for h in range(2):
                                nc.tensor.transpose(
                                    pt[:, tt, h, :],
                                    fb[:, 2 * g4 + tt, h * P:(h + 1) * P],
                                    identb[:])
                        dst = fT[:, :, 2 * P * g4: 2 * P * (g4 + 1)]
                        dst = dst.rearrange("p h (tt r) -> p tt h r", tt=2)
                        nc.scalar.activation(dst, pt[:],
                                             mybir.ActivationFunctionType.Copy)

                    # matvec: gl[2, T*128] += w_half.T @ fT_half
                    glp = ppGl.tile([2, T * P], F32)
                    for rc in range(T * P // 512):
                        for h in range(2):
                            nc.tensor.matmul(
                                glp[:, rc * 512:(rc + 1) * 512],
                                wsb[:, 2 * h:2 * h + 2],
                                fT[:, h, rc * 512:(rc + 1) * 512],
                                start=(h == 0), stop=(h == 1))

                    gl_sb = pGl.tile([2, T * P], F32)
                    if GL_BOUNCE == "dma":
                        nc.sync.dma_start(gl_sb[:], glp[:])
                    else:
                        nc.scalar.activation(gl_sb[:], glp[:],
                                             mybir.ActivationFunctionType.Copy)

                    # transpose gl -> [128, T, 2]
                    glt = ppGt.tile([P, T, 2], F32)
                    for t in range(T):
                        nc.tensor.transpose(glt[:, t, :],
                                            gl_sb[:, t * P:(t + 1) * P],
                                            identf[:])

                    # e = exp(g + bg); em = exp(-(l + bl)); u = e/(1+em)
                    # (sigmoid via the exp table only -- avoids the ~1.3us
                    #  ACT table reload on every exp<->sigmoid switch)
                    e_bf = pSm.tile([P, T], BF16, tag="e")
                    nc.scalar.activation(e_bf[:], glt[:, :, 0],
                                         mybir.ActivationFunctionType.Exp,
                                         bias=bias[:, 0:1])
                    em_bf = pSm.tile([P, T], BF16, tag="em")
                    nc.scalar.activation(em_bf[:], glt[:, :, 1],
                                         mybir.ActivationFunctionType.Exp,
                                         bias=bias[:, 1:2], scale=-1.0)
                    op_f = pSm.tile([P, T], F32, tag="op")
                    nc.vector.tensor_scalar_add(op_f[:], em_bf[:], 1.0)
                    rc_f = pSm.tile([P, T], F32, tag="rc")
                    nc.vector.reciprocal(rc_f[:], op_f[:])
                    u_bf = pSm.tile([P, T], BF16, tag="u")
                    nc.vector.tensor_tensor(u_bf[:], e_bf[:], rc_f[:],
                                            mybir.AluOpType.mult)

                    H = pSm.tile([P, T, B], BF16, tag="H")
                    nc.vector.tensor_tensor(
                        H[:],
                        segs[:, m * T:(m + 1) * T].unsqueeze(2).to_broadcast((P, T, B)),
                        iota[:].unsqueeze(1).to_broadcast((P, T, B)),
                        mybir.AluOpType.is_equal)
                    Hu = pSm.tile([P, T, B], BF16, tag="Hu")
                    nc.vector.tensor_tensor(
                        Hu[:], H[:],
                        u_bf[:].unsqueeze(2).to_broadcast((P, T, B)),
                        mybir.AluOpType.mult)

                    for t in range(T):
                        tg = m * T + t
                        st = (tg == 0)
                        sp = (tg == tiles - 1)
                        nc.tensor.matmul(ppAB[:, 0:C], Hu[:, t, :], fb[:, t, :],
                                         start=st, stop=sp, skip_group_check=True)
                        nc.tensor.matmul(ppAB[:, C:2 * C], Hu[:, t, :], f2b[:, t, :],
                                         start=st, stop=sp, skip_group_check=True)
                        nc.tensor.matmul(ppS[:, 0:1], Hu[:, t, :], onesb[:],
                                         start=st, stop=sp, skip_group_check=True)
                        nc.tensor.matmul(ppS[:, 1:2], H[:, t, :], e_bf[:, t:t + 1],
                                         start=st, stop=sp, skip_group_check=True)

            # =================== collective + finals ===================
            with (
                tc.tile_pool(name="ep", bufs=1) as pe,
                tc.tile_pool(name="ppE", bufs=1, space="PSUM") as ppE,
            ):
                ABs = pe.tile([B, 2 * C + 2], F32)
                nc.scalar.activation(ABs[:, 0:2 * C], ppAB[:],
                                     mybir.ActivationFunctionType.Copy)
                nc.scalar.activation(ABs[:, 2 * C:2 * C + 2], ppS[:],
                                     mybir.ActivationFunctionType.Copy)

                cc_in = pdram.tile([B, 2 * C + 2], F32)
                cc_out = pdram.tile([B, 2 * C + 2], F32)
                nc.sync.dma_start(cc_in[:], ABs[:])
                nc.gpsimd.collective_compute(
                    "AllReduce", mybir.AluOpType.add,
                    replica_groups=[list(range(NCORES))],
                    ins=[cc_in.opt()], outs=[cc_out.opt()])
                R = pe.tile([B, 2 * C + 2], F32)
                nc.sync.dma_start(R[:], cc_out[:])

                # rs = 1/(z+tiny) ; scale [A|B|s] rows by rs (tiny keeps
                # empty segments at 0 contribution instead of 0*inf=NaN)
                zs = pe.tile([B, 1], F32, tag="zs")
                nc.vector.tensor_scalar_add(zs[:], R[:, 2 * C + 1:2 * C + 2], 1e-30)
                rs = pe.tile([B, 1], F32)
                nc.vector.reciprocal(rs[:], zs[:])
                ABn = pe.tile([B, 2 * C + 1], F32)
                nc.vector.tensor_scalar_mul(ABn[:], R[:, 0:2 * C + 1], rs[:])

                # column-sum over the 16 segment partitions via matmul
                tot = ppE.tile([1, 2 * C], F32)
                nc.tensor.matmul(tot[0:1, :], onesf[:, 0:1], ABn[:, 0:2 * C],
                                 start=True, stop=True, skip_group_check=True)
                tot2 = ppE.tile([1, 1], F32, tag="tot2")
                nc.tensor.matmul(tot2[0:1, 0:1], onesf[:, 0:1], ABn[:, 2 * C:2 * C + 1],
                                 start=True, stop=True, skip_group_check=True)

                sinv = pe.tile([1, 1], F32)
                nc.vector.reciprocal(sinv[:], tot2[0:1, 0:1])
                fin = pe.tile([1, 2 * C], F32)   # [mean | E2]
                nc.vector.tensor_scalar_mul(fin[:], tot[0:1, :], sinv[:])

                mean2 = pe.tile([1, C], F32)
                nc.vector.tensor_tensor(mean2[:], fin[:, 0:C], fin[:, 0:C],
                                        mybir.AluOpType.mult)
                var = pe.tile([1, C], F32)
                nc.vector.tensor_tensor(var[:], fin[:, C:2 * C], mean2[:],
                                        mybir.AluOpType.subtract)
                stdv = pe.tile([1, C], F32)
                nc.scalar.activation(stdv[:], var[:],
                                     mybir.ActivationFunctionType.Sqrt)
                mr = pe.tile([1, 2 * C], F32)    # [mean*rstd | rstd]
                nc.vector.reciprocal(mr[:, C:2 * C], stdv[:])
                nc.vector.tensor_tensor(mr[:, 0:C], fin[:, 0:C], mr[:, C:2 * C],
                                        mybir.AluOpType.mult)

                # replicate [1, 512] -> [128, 512] via K=1 matmul with ones row
                rep = ppE.tile([P, 2 * C], F32, tag="rep")
                nc.tensor.matmul(rep[:], onesf[0:1, :], mr[:],
                                 start=True, stop=True, skip_group_check=True)
                mrr = pfin.tile([P, 2 * C], F32)
                nc.scalar.activation(mrr[:], rep[:],
                                     mybir.ActivationFunctionType.Copy)

            # =================== PASS 2: normalize ===================
            with (
                tc.tile_pool(name="pF2", bufs=4) as pF2,
                tc.tile_pool(name="pO", bufs=4) as pO,
            ):
                mmul_b = mrr[:, 0:C].unsqueeze(1).to_broadcast((P, T, C))
                rstd_b = mrr[:, C:2 * C].unsqueeze(1).to_broadcast((P, T, C))
                for m in range(macros):
                    f_t = pF2.tile([P, T, C], F32)
                    nc.sync.dma_start(
                        f_t[:], feats_d[m * T:(m + 1) * T].rearrange("t p c -> p t c"))
                    o1 = pO.tile([P, T, C], F32, tag="o1")
                    nc.vector.tensor_tensor(o1[:], f_t[:], rstd_b,
                                            mybir.AluOpType.mult)
                    o2 = pO.tile([P, T, C], F32, tag="o2")
                    nc.vector.tensor_tensor(o2[:], o1[:], mmul_b,
                                            mybir.AluOpType.subtract)
                    nc.sync.dma_start(
                        out_d[m * T:(m + 1) * T].rearrange("t p c -> p t c"), o2[:])

    nc.compile()
    return nc


def _prep_inputs(feats, segment_ids, w_local, b_local, w_global, b_global):
    n, c = feats.shape
    assert c == C
    rows_core = (n + NCORES - 1) // NCORES
    macros = (rows_core + T * P - 1) // (T * P)
    tiles = macros * T
    rows_pad = tiles * P

    wcat = np.concatenate([w_global.reshape(C, 1), w_local.reshape(C, 1)], axis=1)
    wsb = wcat.reshape(2, P, 2).transpose(1, 0, 2).reshape(P, 4)  # [c, 2h+j]
    bias2 = np.tile(np.array([b_global[0], -b_local[0]], np.float32), (P, 1))

    in_maps = []
    for k in range(NCORES):
        lo, hi = k * rows_core, min((k + 1) * rows_core, n)
        fs = np.zeros((rows_pad, C), np.float32)
        fs[:hi - lo] = feats[lo:hi]
        ss = np.full((rows_pad,), -1.0, np.float32)
        ss[:hi - lo] = segment_ids[lo:hi].astype(np.float32)
        in_maps.append({
            "feats": np.ascontiguousarray(fs.reshape(tiles, P, C)),
            "segs": np.ascontiguousarray(
                ss.reshape(tiles, P).T.astype(ml_dtypes.bfloat16)),
            "wsb": wsb.astype(ml_dtypes.bfloat16),
            "bias2": bias2.astype(np.float32),
            "identb": np.eye(P, dtype=ml_dtypes.bfloat16),
            "identf": np.eye(2, dtype=np.float32),
            "iotab": np.tile(np.arange(B, dtype=np.float32), (P, 1)).astype(
                ml_dtypes.bfloat16),
            "onesb": np.ones((P, 1), ml_dtypes.bfloat16),
            "onesf": np.ones((B, P), np.float32),
        })
    return in_maps, tiles, rows_core


_CACHE = {}


def _run(in_maps, tiles, **kw):
    if tiles not in _CACHE:
        _CACHE[tiles] = build_graph(tiles)
    nc = _CACHE[tiles]
    return run_bass_kernel_spmd(nc, in_maps, core_ids=list(range(NCORES)), **kw)


def kernel(feats, segment_ids, w_local, b_local, w_global, b_global,
           _return_results=False, **run_kw):
    feats = np.asarray(feats, np.float32)
    segment_ids = np.asarray(segment_ids)
    in_maps, tiles, rows_core = _prep_inputs(
        feats, segment_ids,
        np.asarray(w_local, np.float32), np.asarray(b_local, np.float32),
        np.asarray(w_global, np.float32), np.asarray(b_global, np.float32))
    res = _run(in_maps, tiles, **run_kw)
    n = feats.shape[0]
    outs = []
    for k in range(NCORES):
        lo, hi = k * rows_core, min((k + 1) * rows_core, n)
        o = res.results[k]["out"].reshape(-1, C)[:hi - lo]
        outs.append(o)
    full = np.concatenate(outs, axis=0)
    if _return_results:
        return full, res
    return full
